# revision 1
# baseline (speedup 1.0000x reference)
"""ExtraMSAEmbedding Trainium2 kernel.

out[s, r, :] = one_hot(msa[s, r], 23) @ W[:, :23].T
             + has_del[s, r] * W[:, 23] + del_val[s, r] * W[:, 24] + b

Strategy (8 NeuronCores, data-parallel over the 2048 extra sequences — 256
seqs = 98304 tokens per core):

- tokens are processed in blocks of 512 (one PSUM bank), 4 blocks
  ("groups" g=0..3) per iteration, SUPER=8 iterations per DMA batch.
- a K=1 matmul on the TensorEngine broadcasts each block's msa values (as
  f32) across 32 PSUM partitions (lhsT is a 0/1 mask row so rows >= 23 get
  0.0)
- one DVE tensor_scalar(is_equal) per iteration against a per-partition
  class-index column turns that into the transposed one-hot
  ([class, token] layout).  Partition row 25 compares 0==0 -> constant
  1.0 (bias row); rows 23/24 are overwritten by DMA of has_del/del_val
  (batched once per super-block).
- the embedding is a single K=26 matmul per block with stationary weights
  [W.T ; b] producing out.T tiles [64 channels, 512 tokens]. The 4 block
  matmuls + 4 broadcast matmuls of an iteration sit on disjoint 32x32 PE
  subarrays via tile_position, so they overlap on the array.
- ScalarE (ACT) copies PSUM->SBUF into big [128, SUPER*512] staging
  tiles; outputs leave as raw [iter, bank, 128, 512] dumps via SWDGE
  (gpsimd) DMA — descriptors spread over all 16 SDMA engines.  The host
  does the final (cheap) layout transpose while unsharding.
"""

import numpy as np

N_SEQ, N_RES = 2048, 384
C_OUT = 64
N_CORES = 8
SEQ_PER_CORE = N_SEQ // N_CORES  # 256
T_PER_CORE = SEQ_PER_CORE * N_RES  # 98304
BLK = 512  # tokens per block (one PSUM bank of f32)
N_BLOCKS = T_PER_CORE // BLK  # 192
GROUPS = 4  # blocks per iteration
SUPER = 8  # iterations per DMA batch
# group g's msa staging row lives at partition 32*PI[g] (chosen so the
# broadcast matmuls land on PE subarrays disjoint from the main matmuls)
PI = [(g + 2) % 4 for g in range(GROUPS)]

_CACHE: dict = {}
_LAST_RESULT = None


def build_program(n_blocks: int = N_BLOCKS):
    """Build + compile the Bass/Tile program (same program for all cores)."""
    import concourse.bass as bass  # noqa: F401
    import concourse.mybir as mybir
    import concourse.tile as tile
    from concourse import bacc

    f32 = mybir.dt.float32
    bf16 = mybir.dt.bfloat16
    assert n_blocks % (GROUPS * SUPER) == 0
    n_super = n_blocks // (GROUPS * SUPER)
    FREE = SUPER * BLK  # free-dim of the big per-super tiles

    nc = bacc.Bacc("TRN2", target_bir_lowering=False, debug=False)

    # inputs laid out per super-block by the host (see kernel() below)
    # msa in bf16: exact for integers 0..22, makes the broadcast matmul a
    # single-pass bf16 matmul instead of a two-pass fp32 one
    msa_d = nc.dram_tensor(
        "msa", [n_super, GROUPS, SUPER, BLK], bf16, kind="ExternalInput"
    ).ap()
    # has_del / del_val, each split into 3 bf16 components on the host
    # (h1+h2+h3 == fp32 value exactly); 6 planes total, feat rows 23..28
    # (one contiguous-partition DMA per 32-row group)
    hd_d = nc.dram_tensor(
        "hd", [n_super, GROUPS, 6, SUPER, BLK], bf16, kind="ExternalInput"
    ).ap()
    # stationary weights: three bf16 components of [W.T classes; w23 x3;
    # w24 x3; b] so the K=30 matmul runs as 3 accumulating bf16 passes
    # (packed side by side in the free dim: [128, 3*C_OUT])
    w30_d = nc.dram_tensor("w30", [128, 3 * C_OUT], bf16, kind="ExternalInput").ap()
    mask_d = nc.dram_tensor("mask", [128, 32], bf16, kind="ExternalInput").ap()
    ccol_d = nc.dram_tensor("ccol", [128, 1], f32, kind="ExternalInput").ap()
    # raw output dump: [super, 128 partitions, SUPER iters, 1024] -> per
    # partition each super-store is one contiguous 32 KB run (host fixes
    # the layout when unsharding)
    out_d = nc.dram_tensor(
        "out", [n_super, 128, SUPER, 2 * BLK], f32, kind="ExternalOutput"
    ).ap()

    with tile.TileContext(nc) as tc:
        with (
            # consts pool created after the big pools: the simulator models
            # bf16 weight loads with a widened read span, which must not
            # overlap the next-allocated tensor
            tc.tile_pool(name="staging", bufs=3) as spool,
            tc.tile_pool(name="feat", bufs=3) as fpool,
            tc.tile_pool(name="osb", bufs=3) as opool,
            tc.tile_pool(name="consts", bufs=1) as cpool,
            tc.tile_pool(name="pbc", bufs=2, space=bass.MemorySpace.PSUM) as pbpool,
            tc.tile_pool(name="pout", bufs=3, space=bass.MemorySpace.PSUM) as popool,
        ):
            # const loads on the Scalar HWDGE ring so the first msa staging
            # DMA isn't queued behind them on Sync
            w30 = cpool.tile([128, 3 * C_OUT], bf16)
            nc.scalar.dma_start(w30[:], w30_d)
            mask = cpool.tile([128, 32], bf16)
            nc.scalar.dma_start(mask[:], mask_d)
            ccol = cpool.tile([128, 1], f32)
            nc.scalar.dma_start(ccol[:], ccol_d)

            for s in range(n_super):
                # big input staging: partition 32p holds msa of group (p+2)%4
                # for the 8 iterations of this super-block
                staging = spool.tile([128, FREE], bf16)
                nc.sync.dma_start(staging[0:128:32, :], msa_d[s])

                feat = fpool.tile([128, FREE], bf16)
                for j in range(SUPER):
                    cs = slice(j * BLK, (j + 1) * BLK)
                    pb = pbpool.tile([128, BLK], f32, name="pb")
                    # broadcast matmuls: pb[32g+k, t] = mask[k]*msa_g[t]
                    for g in range(GROUPS):
                        pg = 32 * PI[g]
                        nc.tensor.matmul(
                            pb[32 * g : 32 * g + 32, :],
                            mask[pg : pg + 1, :],
                            staging[pg : pg + 1, cs],
                            tile_position=(pg, 32 * g),
                        )
                    # one-hot (+ ones row 29) via is_equal vs class column
                    nc.vector.tensor_scalar(
                        feat[:, cs], pb[:], ccol[:], None, mybir.AluOpType.is_equal
                    )

                # deletion-feature bf16 components into rows 23..28 of each
                # 32-row group (after the eq ops in program order; Tile
                # serializes the overlapping writes correctly).  On the
                # otherwise-idle Sync HWDGE ring: sharing the SWDGE ring
                # with the output stream puts multi-us output drains onto
                # this critical path (measured 1.5x worse).
                for k in range(6):
                    nc.sync.dma_start(feat[23 + k : 128 : 32, :], hd_d[s, :, k, :, :])

                # osb layout per partition: [iter j | bank | 512 tokens]
                osb = opool.tile([128, SUPER * 2 * BLK], f32, name="osb")
                for j in range(SUPER):
                    cs = slice(j * BLK, (j + 1) * BLK)
                    # main matmuls: out.T[64, 512] = W30.T @ feat_g, K=30,
                    # as 3 accumulating bf16 passes (exact fp32 decomp)
                    po = popool.tile([128, 2 * BLK], f32, name="po")
                    for g in range(GROUPS):
                        bank, half = g % 2, 64 * (g // 2)
                        for k in range(3):
                            nc.tensor.matmul(
                                po[half : half + 64, bank * BLK : (bank + 1) * BLK],
                                w30[
                                    32 * g : 32 * g + 30,
                                    k * C_OUT : (k + 1) * C_OUT,
                                ],
                                feat[32 * g : 32 * g + 30, cs],
                                start=(k == 0),
                                stop=(k == 2),
                                tile_position=(32 * g, half),
                            )
                    # PSUM -> SBUF: mostly ACT, 1-in-6 on DVE to balance
                    ocs = slice(j * 2 * BLK, (j + 1) * 2 * BLK)
                    if j % 6 == 5:
                        nc.vector.tensor_copy(osb[:, ocs], po[:])
                    else:
                        nc.scalar.copy(osb[:, ocs], po[:])
                    # raw store via SWDGE (descriptors spread over all 16
                    # SDMA engines), half a super-block at a time
                    if j % (SUPER // 2) == SUPER // 2 - 1:
                        h = j // (SUPER // 2)
                        hs = slice(h * (SUPER // 2), (h + 1) * (SUPER // 2))
                        nc.gpsimd.dma_start(
                            out_d[s, :, hs, :],
                            osb[:, h * FREE : h * FREE + FREE],
                        )

    nc.compile()
    return nc


def _split3(x: np.ndarray) -> np.ndarray:
    """Exact 3-way bf16 decomposition: sum(result) == x (fp32)."""
    import ml_dtypes

    bf = ml_dtypes.bfloat16
    h1 = x.astype(bf)
    r1 = x - h1.astype(np.float32)
    h2 = r1.astype(bf)
    h3 = (r1 - h2.astype(np.float32)).astype(bf)
    return np.stack([h1, h2, h3])


def _host_constants(W: np.ndarray, b: np.ndarray):
    import ml_dtypes

    f32 = np.float32
    # K=30 weight rows: 0-22 classes, 23-25 w23 (x3 has components),
    # 26-28 w24 (x3 del components), 29 bias (ones row)
    w30 = np.zeros((32, C_OUT), f32)
    w30[0:23] = W.T[0:23].astype(f32)
    w30[23:26] = W.T[23].astype(f32)
    w30[26:29] = W.T[24].astype(f32)
    w30[29] = b.astype(f32)
    w30 = np.tile(w30, (4, 1))  # replicate for the 4 K-strips
    # [3, 128, 64] bf16 -> packed [128, 3*64]
    w30_split = np.ascontiguousarray(
        _split3(w30).transpose(1, 0, 2).reshape(128, 3 * C_OUT)
    )

    mask = np.zeros((128, 32), ml_dtypes.bfloat16)
    mask[:, 0:23] = 1.0  # broadcast only class rows; rows 23-31 get 0

    ccol = np.full((128, 1), -7.0, f32)
    for p in range(128):
        j = p % 32
        if j < 23:
            ccol[p] = j  # one-hot compare value
        elif j == 29:
            ccol[p] = 0.0  # matches the broadcast 0 -> constant 1.0 (bias)
    return w30_split, mask, ccol


def _stage_blocks(x_blocks: np.ndarray, perm: bool) -> np.ndarray:
    """[n_blocks, BLK] -> [n_super, GROUPS, SUPER, BLK] staging layout.

    Element [s, p, j] = block 4*(SUPER*s + j) + g  with g = (p+2)%4 when
    perm (msa staging partition order), else g = p (feat row order).
    """
    nb = x_blocks.shape[0]
    x = x_blocks.reshape(nb // (GROUPS * SUPER), SUPER, GROUPS, BLK)
    x = x.transpose(0, 2, 1, 3)  # [s, g, j, t]
    if perm:
        x = x[:, [2, 3, 0, 1], :, :]  # partition p holds group (p+2)%4
    return np.ascontiguousarray(x)


def kernel(extra_msa, extra_has_deletion, extra_deletion_value, W, b):
    from concourse.bass_utils import run_bass_kernel_spmd

    f32 = np.float32
    msa = np.asarray(extra_msa).astype(f32)  # int -> f32 (exact for 0..22)
    has_ = np.asarray(extra_has_deletion, dtype=f32)
    del_ = np.asarray(extra_deletion_value, dtype=f32)
    W = np.asarray(W, dtype=f32)
    b = np.asarray(b, dtype=f32)

    if "nc" not in _CACHE:
        _CACHE["nc"] = build_program(N_BLOCKS)
    nc = _CACHE["nc"]

    w30_split, mask, ccol = _host_constants(W, b)

    import ml_dtypes

    bf = ml_dtypes.bfloat16
    has3 = _split3(has_)  # [3, 2048, 384] bf16 components
    del3 = _split3(del_)

    in_maps = []
    for c in range(N_CORES):
        s0, s1 = c * SEQ_PER_CORE, (c + 1) * SEQ_PER_CORE
        hd = np.stack(
            [
                _stage_blocks(
                    np.ascontiguousarray(x[s0:s1]).reshape(N_BLOCKS, BLK), False
                )
                for x in (has3[0], has3[1], has3[2], del3[0], del3[1], del3[2])
            ],
            axis=2,  # [n_super, GROUPS, 6, SUPER, BLK]
        )
        in_maps.append(
            {
                "msa": _stage_blocks(msa[s0:s1].reshape(N_BLOCKS, BLK), True).astype(
                    bf
                ),
                "hd": hd,
                "w30": w30_split,
                "mask": mask,
                "ccol": ccol,
            }
        )

    res = run_bass_kernel_spmd(nc, in_maps, list(range(N_CORES)))
    global _LAST_RESULT
    _LAST_RESULT = res

    # unshard: raw [super, 128, SUPER, 1024] -> token-major [256, 384, 64]
    n_super = N_BLOCKS // (GROUPS * SUPER)
    parts = []
    for r in res.results:
        raw = r["out"].reshape(n_super, 2, C_OUT, SUPER, 2, BLK)
        # axes (s, half, ch, j, bank, t): block = 4*(SUPER*s+j)+2*half+bank
        tok = raw.transpose(0, 3, 1, 4, 5, 2).reshape(T_PER_CORE, C_OUT)
        parts.append(tok.reshape(SEQ_PER_CORE, N_RES, C_OUT))
    return np.ascontiguousarray(np.concatenate(parts, axis=0))



# revision 7
# speedup vs baseline: 1.3747x; 1.3747x over previous
"""ExtraMSAEmbedding Trainium2 kernel (v2 — all-bf16, single-pass).

out[s, r, :] = one_hot(msa[s, r], 23) @ W[:, :23].T
             + has_del[s, r] * W[:, 23] + del_val[s, r] * W[:, 24] + b

The harness gate is rel_err < 2e-2 against max|out| (=1.37), so the whole
pipeline runs in bf16 (measured end-to-end abs err ~1e-2's tolerance with
>2x margin): bf16 weights/deletion feats, exact bf16 one-hot, f32 PSUM
accumulation, bf16 output.  That halves the dominant HBM write traffic
(12.6 MB/core vs 25.2) and cuts the matmul work 3x vs the exact-fp32
3-pass decomposition.

Strategy (8 NeuronCores, data-parallel over the 2048 sequences — 256 seqs
= 98304 tokens per core, blocks of 512 tokens, 4 blocks per iteration,
8 iterations per super-block, 6 super-blocks):

- msa arrives as bf16 [4, 4096] per super (group g's tokens on partition
  g).  ONE K=4 broadcast matmul per iteration (lhsT = 0/1 block-diagonal
  mask) replicates the 4 groups' msa values onto 4x32 PSUM partitions;
  rows 23..31 of each group get 0.0.
- one DVE tensor_scalar(is_equal) vs a per-partition class column turns
  that into the transposed one-hot ([class, token]).  Row 25 compares
  0==0 -> 1.0 (bias row); rows 23/24 are overwritten by a per-super DMA
  of has_del/del_val (bf16, single plane each).
- TWO main matmuls per iteration (K=64, M=128, FD=512): lhsT w52 holds
  two copies of [W.T(23) ; w23 ; w24 ; b] so po partitions = 2 blocks x
  64 channels.  Single bf16 pass, PSUM f32.
- PSUM -> SBUF bf16 cast copies split ACT/DVE to balance the two engines
  (the 1 elem/lane/cycle PSUM read port is the engine bottleneck).
- outputs leave as raw [super, 128, iter, 1024] bf16 dumps via SWDGE
  (descriptors spread over all 16 SDMA engines); the host does the final
  cheap layout transpose + f32 cast while unsharding.
"""

import numpy as np

N_SEQ, N_RES = 2048, 384
C_OUT = 64
N_CORES = 8
SEQ_PER_CORE = N_SEQ // N_CORES  # 256
T_PER_CORE = SEQ_PER_CORE * N_RES  # 98304
BLK = 512  # tokens per block (one PSUM bank of f32)
N_BLOCKS = T_PER_CORE // BLK  # 192
GROUPS = 4  # blocks per iteration
SUPER = 8  # iterations per super-block

_CACHE: dict = {}
_LAST_RESULT = None


def build_program(n_blocks: int = N_BLOCKS):
    """Build + compile the Bass/Tile program (same program for all cores)."""
    import concourse.bass as bass  # noqa: F401
    import concourse.mybir as mybir
    import concourse.tile as tile
    from concourse import bacc

    f32 = mybir.dt.float32
    bf16 = mybir.dt.bfloat16
    assert n_blocks % (GROUPS * SUPER) == 0
    n_super = n_blocks // (GROUPS * SUPER)
    FREE = SUPER * BLK  # free-dim of the big per-super tiles

    nc = bacc.Bacc("TRN2", target_bir_lowering=False, debug=False)

    # inputs laid out per super-block by the host (see kernel() below)
    # msa bf16 (exact for ints 0..22): partition g holds group g's tokens
    msa_d = nc.dram_tensor(
        "msa", [n_super, GROUPS, SUPER, BLK], bf16, kind="ExternalInput"
    ).ap()
    # has_del / del_val, one bf16 plane each -> feat rows 23, 24
    hd_d = nc.dram_tensor(
        "hd", [n_super, 2, GROUPS, SUPER, BLK], bf16, kind="ExternalInput"
    ).ap()
    # stationary weights: [128, 128] — rows 0:64 feed the h=0 matmul (PE
    # rows 0-63), rows 64:128 the identical copy for h=1 (PE rows 64-127);
    # each half is two K=32 strips of [W.T classes; w23; w24; b]
    w52_d = nc.dram_tensor("w52", [128, 2 * C_OUT], bf16, kind="ExternalInput").ap()
    bmask_d = nc.dram_tensor("bmask", [GROUPS, 128], bf16, kind="ExternalInput").ap()
    ccol_d = nc.dram_tensor("ccol", [128, 1], f32, kind="ExternalInput").ap()
    # raw output dump: [super, 128 partitions, SUPER iters, 1024] bf16 ->
    # per partition each half-super store is one contiguous 8 KB run
    out_d = nc.dram_tensor(
        "out", [n_super, 128, SUPER, 2 * BLK], bf16, kind="ExternalOutput"
    ).ap()

    with tile.TileContext(nc) as tc:
        with (
            tc.tile_pool(name="staging", bufs=3) as spool,
            tc.tile_pool(name="feat", bufs=3) as fpool,
            tc.tile_pool(name="osb", bufs=3) as opool,
            tc.tile_pool(name="consts", bufs=1) as cpool,
            tc.tile_pool(name="pbc", bufs=2, space=bass.MemorySpace.PSUM) as pbpool,
            tc.tile_pool(name="pout", bufs=3, space=bass.MemorySpace.PSUM) as popool,
        ):
            # const loads on the Scalar HWDGE ring so the first msa staging
            # DMA isn't queued behind them on Sync
            w52 = cpool.tile([128, 2 * C_OUT], bf16)
            nc.scalar.dma_start(w52[:], w52_d)
            bmask = cpool.tile([GROUPS, 128], bf16)
            nc.scalar.dma_start(bmask[:], bmask_d)
            ccol = cpool.tile([128, 1], f32)
            nc.scalar.dma_start(ccol[:], ccol_d)

            for s in range(n_super):
                # per-super msa staging: partition g = group g, [4, 4096]
                staging = spool.tile([GROUPS, FREE], bf16)
                nc.sync.dma_start(staging[:], msa_d[s])

                feat = fpool.tile([128, FREE], bf16)
                for j in range(SUPER):
                    cs = slice(j * BLK, (j + 1) * BLK)
                    pb = pbpool.tile([128, BLK], f32, name="pb")
                    # one K=4 broadcast matmul: pb[32g+k, t] = bmask*msa_g[t]
                    nc.tensor.matmul(pb[:, :], bmask[:, :], staging[:, cs])
                    # one-hot (+ ones row 25) via is_equal vs class column
                    nc.vector.tensor_scalar(
                        feat[:, cs], pb[:], ccol[:], None, mybir.AluOpType.is_equal
                    )

                # deletion features (bf16) into rows 23, 24 of each 32-row
                # group (after the eq ops in program order; Tile serializes
                # the overlapping writes).  On the otherwise-idle Sync ring.
                for k in range(2):
                    nc.sync.dma_start(feat[23 + k : 128 : 32, :], hd_d[s, k])

                # osb layout per partition: [iter j | matmul half | 512 tok]
                osb = opool.tile([128, SUPER * 2 * BLK], bf16, name="osb")
                for j in range(SUPER):
                    cs = slice(j * BLK, (j + 1) * BLK)
                    po = popool.tile([128, 2 * BLK], f32, name="po")
                    # main matmuls: po[64c+..] = w52.T @ feat rows, K=64,
                    # M=128 -> 2 blocks per matmul, single bf16 pass
                    for h in range(2):
                        nc.tensor.matmul(
                            po[:, h * BLK : (h + 1) * BLK],
                            w52[64 * h : 64 * h + 64, :],
                            feat[64 * h : 64 * h + 64, cs],
                        )
                    # PSUM -> SBUF bf16 cast: balance ACT vs DVE (DVE also
                    # carries the 8 eq ops per super): ~7 ACT / 1 DVE
                    ocs = slice(j * 2 * BLK, (j + 1) * 2 * BLK)
                    if j % 8 == 3:
                        nc.vector.tensor_copy(osb[:, ocs], po[:])
                    else:
                        nc.scalar.copy(osb[:, ocs], po[:])
                    # raw store via SWDGE (descriptors spread over all 16
                    # SDMA engines), half a super-block at a time
                    if j % (SUPER // 2) == SUPER // 2 - 1:
                        h2 = j // (SUPER // 2)
                        hs = slice(h2 * (SUPER // 2), (h2 + 1) * (SUPER // 2))
                        nc.gpsimd.dma_start(
                            out_d[s, :, hs, :],
                            osb[:, h2 * FREE : h2 * FREE + FREE],
                        )

    nc.compile()
    return nc


def _host_constants(W: np.ndarray, b: np.ndarray):
    import ml_dtypes

    bf = ml_dtypes.bfloat16
    f32 = np.float32
    # two K=32 strips of [W.T classes(23); w23; w24; b], for feat groups
    # (g, g+1) -> output channels [block even | block odd]
    w26 = np.zeros((32, C_OUT), f32)
    w26[0:23] = W.T[0:23].astype(f32)
    w26[23] = W.T[23].astype(f32)
    w26[24] = W.T[24].astype(f32)
    w26[25] = b.astype(f32)
    w52 = np.zeros((64, 2 * C_OUT), f32)
    w52[0:32, 0:C_OUT] = w26
    w52[32:64, C_OUT : 2 * C_OUT] = w26
    w52 = np.tile(w52, (2, 1)).astype(bf)  # rows 64:128 = copy for h=1

    bmask = np.zeros((GROUPS, 128), bf)
    for k in range(GROUPS):
        bmask[k, 32 * k : 32 * k + 23] = 1.0

    ccol = np.full((128, 1), -7.0, f32)
    for p in range(128):
        j = p % 32
        if j < 23:
            ccol[p] = j  # one-hot compare value
        elif j == 25:
            ccol[p] = 0.0  # matches the broadcast 0 -> constant 1.0 (bias)
    return w52, bmask, ccol


def _stage_blocks(x_blocks: np.ndarray) -> np.ndarray:
    """[n_blocks, BLK] -> [n_super, GROUPS, SUPER, BLK] staging layout.

    Element [s, g, j] = flat block 4*(SUPER*s + j) + g.
    """
    nb = x_blocks.shape[0]
    x = x_blocks.reshape(nb // (GROUPS * SUPER), SUPER, GROUPS, BLK)
    return np.ascontiguousarray(x.transpose(0, 2, 1, 3))


def kernel(extra_msa, extra_has_deletion, extra_deletion_value, W, b):
    from concourse.bass_utils import run_bass_kernel_spmd

    import ml_dtypes

    bf = ml_dtypes.bfloat16
    f32 = np.float32
    msa = np.asarray(extra_msa).astype(f32)  # int -> f32 (exact for 0..22)
    has_ = np.asarray(extra_has_deletion, dtype=f32).astype(bf)
    del_ = np.asarray(extra_deletion_value, dtype=f32).astype(bf)
    W = np.asarray(W, dtype=f32)
    b = np.asarray(b, dtype=f32)

    if "nc" not in _CACHE:
        _CACHE["nc"] = build_program(N_BLOCKS)
    nc = _CACHE["nc"]

    w52, bmask, ccol = _host_constants(W, b)

    in_maps = []
    for c in range(N_CORES):
        s0, s1 = c * SEQ_PER_CORE, (c + 1) * SEQ_PER_CORE
        hd = np.stack(
            [
                _stage_blocks(np.ascontiguousarray(x[s0:s1]).reshape(N_BLOCKS, BLK))
                for x in (has_, del_)
            ],
            axis=1,  # [n_super, 2, GROUPS, SUPER, BLK]
        )
        in_maps.append(
            {
                "msa": _stage_blocks(msa[s0:s1].reshape(N_BLOCKS, BLK)).astype(bf),
                "hd": hd,
                "w52": w52,
                "bmask": bmask,
                "ccol": ccol,
            }
        )

    res = run_bass_kernel_spmd(nc, in_maps, list(range(N_CORES)))
    global _LAST_RESULT
    _LAST_RESULT = res

    # unshard: raw [super, 128, SUPER, 1024] bf16 -> token-major f32
    n_super = N_BLOCKS // (GROUPS * SUPER)
    parts = []
    for r in res.results:
        raw = np.asarray(r["out"]).reshape(n_super, 2, C_OUT, SUPER, 2, BLK)
        # axes (s, phalf, ch, j, half, t): block = 4*(8s+j) + 2*half + phalf
        tok = raw.transpose(0, 3, 4, 1, 5, 2).reshape(T_PER_CORE, C_OUT)
        parts.append(tok.astype(f32).reshape(SEQ_PER_CORE, N_RES, C_OUT))
    return np.ascontiguousarray(np.concatenate(parts, axis=0))
